# revision 23
# baseline (speedup 1.0000x reference)
"""Mimi-style GQA attention (RoPE + 250-wide sliding causal window) on 8 TRN2 NeuronCores.

Sharding (per spec hint): core c handles batch b=c//4 and KV-head group g=c%4
(4 query heads + 1 KV head). Wq/Wk/Wv column-sharded, Wo row-sharded along the
head dim; each core emits a partial [S, D] output (f16); host sums the 4
partials per batch.

The PE executes its queue in order, so emission order is the schedule: phase 1
emits both projection sweeps' matmuls before any rope chain, and phase 2
software-pipelines strips ([scores(s)] [outproj(s-1)] [PV(s)] [chain(s)]) so
the normalize chain latency is covered by ready matmul work instead of
head-of-line blocking the PE (which re-throttles HAM).

Per-core pipeline (matmul inputs fp16, PSUM accumulation fp32):
  1. Fused [Wv|Wk] + Wq projections from host-pre-transposed hsT, dt-outer in
     two 2-strip sweeps over all 3 groups so the PE tracks the hsT DMA stream
     (split over both HWDGE rings). RoPE as raw*cos + (PERM @ raw)*sin_signed;
     K computed on partitions 64:128 and duplicated to both halves with a
     block-dup matmul.
  2. Banded attention, scoresT [k, q]: per (k-tile, head-pair) one
     [128, 2, 512] PSUM tile (bank-aligned), the two heads' matmuls run
     concurrently on PE row-groups (0,0)/(64,0); one Exp per pair; band mask
     multiply alternates DVE/GpSimd. PV accumulates per 512-wide q-strip into
     [65, 2, 512] PSUM (row 64 = softmax denominators via ones column); bank
     cleared by a 1-column start=True seed, evacuated to SBUF immediately.
  3. Normalize: reciprocal_approx_fast on a DMA-reshaped [16, 128] layout,
     DMA partition-broadcast (stride-0 source), DVE multiplies; row-sharded
     Wo projection emits the f16 partial output.
"""
import os
import sys

for _p in ("/opt/trn_rl_repo", "/root/.axon_site/_ro/trn_rl_repo"):
    if os.path.isdir(_p) and _p not in sys.path:
        sys.path.append(_p)

import numpy as np
import concourse.bass as bass
import concourse.mybir as mybir
import concourse.tile as tile
from concourse import bacc
from concourse.bass_utils import run_bass_kernel_spmd
from concourse.masks import make_identity

F32 = mybir.dt.float32
F16 = mybir.dt.float16
AF = mybir.ActivationFunctionType
OP = mybir.AluOpType

B, S, D = 2, 2048, 1024
H, HK, HD = 16, 4, 64
WINDOW = 250
SCALE = 1.0 / np.sqrt(HD)
THETA = 10000.0
NKT = S // 128          # 16 k-tiles
NST = S // 512          # 4 q-strips
WIN = 384               # padded per-k-tile q-window


def _pv_pieces(s):
    """PV pieces for q-strip s: list of (kt, c0, c1) window-column ranges."""
    out = []
    for kt in range(max(0, 4 * s - 2), min(NKT - 1, 4 * s + 3) + 1):
        j0 = 128 * kt
        w = min(WIN, S - j0)
        c_lo = max(0, 512 * s - j0)
        c_hi = min(w, 512 * (s + 1) - j0)
        if c_lo < c_hi:
            out.append((kt, c_lo, c_hi))
    return out


def _build():
    nc = bacc.Bacc(None, target_bir_lowering=False)

    hsT = nc.declare_dram_parameter("hsT", [8, 128, S], F16, isOutput=False)
    wq = nc.declare_dram_parameter("wqT", [8, 128, 256], F16, isOutput=False)
    wkv = nc.declare_dram_parameter("wkvT", [8, 128, 128], F16, isOutput=False)
    wo = nc.declare_dram_parameter("woT", [2, 128, D], F16, isOutput=False)
    cosd = nc.declare_dram_parameter("cos2", [128, S], F16, isOutput=False)
    sind = nc.declare_dram_parameter("sinS2", [128, S], F16, isOutput=False)
    maskd = nc.declare_dram_parameter("bandmask", [128, WIN], F16, isOutput=False)
    permd = nc.declare_dram_parameter("permT", [128, 128], F16, isOutput=False)
    dupd = nc.declare_dram_parameter("dup64", [64, 128], F16, isOutput=False)
    o_part = nc.declare_dram_parameter("o_part", [S, D], F16, isOutput=True)

    with tile.TileContext(nc) as tc:
        with (
            tc.tile_pool(name="persist", bufs=1) as pp,
            tc.tile_pool(name="work", bufs=4) as wk_pool,
            tc.tile_pool(name="expm", bufs=14) as ep,
            tc.tile_pool(name="norm", bufs=2) as npool,
            tc.tile_pool(name="ost", bufs=4) as opool,
        ):
            # ---- constants needing no DMA: warmup + exp-table preload ----
            warm = pp.tile([128, 512], F16, tag="warm")
            nc.vector.memset(warm, 0.0)
            nc.scalar.activation(warm[0:1, 0:8], warm[0:1, 0:8], AF.Exp, scale=1.0)
            zero65 = pp.tile([128, 65], F16, tag="zero65")
            nc.vector.memset(zero65, 0.0)
            ones1 = pp.tile([1, 64], F16, tag="ones1")
            nc.vector.memset(ones1, 1.0)
            ident = pp.tile([64, 64], F16, tag="ident")
            make_identity(nc, ident)

            # ---- DMA loads: small weights first, hsT split over both rings ----
            wq_sb = pp.tile([128, 8, 256], F16, tag="wq")
            wkv_sb = pp.tile([128, 8, 128], F16, tag="wkv")
            perm_sb = pp.tile([128, 128], F16, tag="perm")
            dup_sb = pp.tile([128, 128], F16, tag="dup")
            mask_sb = pp.tile([128, WIN], F16, tag="mask")
            ht = [pp.tile([128, S], F16, tag=f"ht{dt}", name=f"ht{dt}")
                  for dt in range(8)]
            # sync ring: wkv, ht0 first so group-0 dt-0 matmuls start earliest
            nc.sync.dma_start(out=wkv_sb, in_=wkv.rearrange("a p c -> p a c"))
            nc.sync.dma_start(out=ht[0][:, 0:1024], in_=hsT[0][:, 0:1024])
            nc.sync.dma_start(out=ht[0][:, 1024:2048], in_=hsT[0][:, 1024:2048])
            nc.sync.dma_start(out=wq_sb, in_=wq.rearrange("a p c -> p a c"))
            # scalar ring: small constants then odd ht tiles
            nc.scalar.dma_start(out=perm_sb, in_=permd[:, :])
            nc.scalar.dma_start(out=dup_sb[64:128, :], in_=dupd[:, :])
            nc.scalar.dma_start(out=mask_sb, in_=maskd[:, :])
            for dt in range(1, 8):
                eng = nc.sync if dt % 2 == 0 else nc.scalar
                eng.dma_start(out=ht[dt][:, 0:1024], in_=hsT[dt][:, 0:1024])
                eng.dma_start(out=ht[dt][:, 1024:2048], in_=hsT[dt][:, 1024:2048])
            wo_sb = pp.tile([128, 2, D], F16, tag="wo")
            nc.gpsimd.dma_start(out=wo_sb, in_=wo.rearrange("a p c -> p a c"))
            cos_sb = pp.tile([128, S], F16, tag="cos")
            sin_sb = pp.tile([128, S], F16, tag="sin")
            nc.gpsimd.dma_start(out=cos_sb, in_=cosd[:, :])
            nc.gpsimd.dma_start(out=sin_sb, in_=sind[:, :])

            qT = [pp.tile([128, S], F16, tag=f"qT{m}", name=f"qT{m}") for m in range(2)]
            kdup = pp.tile([128, S], F16, tag="kdup")
            vaug = [pp.tile([128, 4, 65], F16, tag=f"vaug{sp}", name=f"vaug{sp}")
                    for sp in range(NST)]
            aT = [pp.tile([128, S], F16, tag=f"aT{m}", name=f"aT{m}") for m in range(2)]

            # ---- phase 1: projections + rope ----
            # Emission: [sweepA MMs][A raw copies][sweepB MMs][A ropes][B copies][B ropes]
            with (
                tc.tile_pool(name="ps1", bufs=6, space="PSUM") as ps1,
                tc.tile_pool(name="ps1b", bufs=1, space="PSUM") as ps1b,
            ):
                wps = ps1b.tile([128, 512], F32, tag="scr", name="warmps")
                for _w in range(16):
                    nc.tensor.matmul(wps, warm[:, 0:128], warm,
                                     start=True, stop=True)

                def rope_q(m, strip, raw):
                    """qT[m][:, strip] = raw*cos + (PERM @ raw)*sin_signed."""
                    sl = bass.ts(strip, 512)
                    rot = ps1b.tile([128, 512], F32, tag="scr", name=f"rotq{m}_{strip}")
                    nc.tensor.matmul(rot, perm_sb, raw, start=True, stop=True)
                    t1 = wk_pool.tile([128, 512], F16, tag="t1q")
                    nc.vector.tensor_tensor(out=t1, in0=rot, in1=sin_sb[:, sl],
                                            op=OP.mult)
                    t2 = wk_pool.tile([128, 512], F16, tag="t2q")
                    nc.gpsimd.tensor_tensor(out=t2, in0=raw, in1=cos_sb[:, sl],
                                            op=OP.mult)
                    nc.vector.tensor_tensor(out=qT[m][:, sl], in0=t1, in1=t2,
                                            op=OP.add)

                def consume_kv(strip, raw):
                    """V rows 0:64 -> vaug transposes; K rows 64:128 -> rope -> kdup."""
                    sl = bass.ts(strip, 512)
                    # V: 4 transposes into one psum tile, one batched copy out
                    pvt = ps1b.tile([128, 256], F16, tag="vtr", name=f"pvt{strip}")
                    for i in range(4):
                        nc.tensor.transpose(pvt[:, bass.ts(i, 64)],
                                            raw[0:64, bass.ts(i, 128)], ident)
                    vt = vaug[strip]
                    nc.vector.tensor_copy(
                        vt[:, :, 0:64],
                        bass.AP(tensor=pvt.tensor, offset=pvt.offset,
                                ap=[list(pvt.ap[0]), [64, 4], [1, 64]]))
                    nc.vector.memset(vt[:, :, 64:65], 1.0)
                    # K rope on partitions 64:128
                    rot = ps1b.tile([128, 512], F32, tag="scr", name=f"rotk{strip}")
                    nc.tensor.matmul(rot[64:128, :], perm_sb[64:128, 64:128],
                                     raw[64:128, :], start=True, stop=True,
                                     tile_position=(64, 64))
                    t1 = wk_pool.tile([128, 512], F16, tag="t1k")
                    nc.vector.tensor_tensor(out=t1[64:128, :], in0=rot[64:128, :],
                                            in1=sin_sb[64:128, sl], op=OP.mult)
                    t2 = wk_pool.tile([128, 512], F16, tag="t2k")
                    nc.gpsimd.tensor_tensor(out=t2[64:128, :], in0=raw[64:128, :],
                                            in1=cos_sb[64:128, sl], op=OP.mult)
                    kroped = wk_pool.tile([128, 512], F16, tag="krop")
                    nc.vector.tensor_tensor(out=kroped[64:128, :], in0=t1[64:128, :],
                                            in1=t2[64:128, :], op=OP.add)
                    dps = ps1b.tile([128, 512], F32, tag="scr", name=f"dup{strip}")
                    nc.tensor.matmul(dps, dup_sb[64:128, :], kroped[64:128, :],
                                     start=True, stop=True, tile_position=(64, 0))
                    nc.scalar.copy(kdup[:, sl], dps)

                def lhs_of(g, dt):
                    if g == 0:
                        return wkv_sb[:, dt]
                    return wq_sb[:, dt, bass.ts(g - 1, 128)]

                raws = {}

                def emit_mms(sweep):
                    strips = (2 * sweep, 2 * sweep + 1)
                    pst = {(g, sp): ps1.tile([128, 512], F32, tag="proj",
                                             name=f"pj{g}_{sp}")
                           for g in range(3) for sp in strips}
                    for dt in range(8):
                        if sweep == 0 and dt >= 2:
                            # keep-warm filler while the hsT DMA stream catches up
                            fil = ps1b.tile([128, 512], F32, tag="scr",
                                            name=f"fil{dt}")
                            for _w in range(3):
                                nc.tensor.matmul(fil, warm[:, 0:128], warm,
                                                 start=True, stop=True)
                        for g in range(3):
                            for sp in strips:
                                nc.tensor.matmul(pst[(g, sp)], lhs_of(g, dt),
                                                 ht[dt][:, bass.ts(sp, 512)],
                                                 start=(dt == 0), stop=(dt == 7))
                    return pst

                def emit_copies(sweep, pst):
                    strips = (2 * sweep, 2 * sweep + 1)
                    for g in range(3):
                        for sp in strips:
                            raw = wk_pool.tile([128, 512], F16, tag="raw")
                            nc.scalar.copy(raw, pst[(g, sp)])
                            raws[(g, sp)] = raw

                def emit_ropes(sweep):
                    strips = (2 * sweep, 2 * sweep + 1)
                    for g in range(3):
                        for sp in strips:
                            if g == 0:
                                consume_kv(sp, raws[(g, sp)])
                            else:
                                rope_q(g - 1, sp, raws[(g, sp)])

                pstA = emit_mms(0)
                emit_copies(0, pstA)
                pstB = emit_mms(1)
                emit_ropes(0)
                emit_copies(1, pstB)
                emit_ropes(1)

            # ---- phases 2+3: banded attention + output projection ----
            # per strip: [scores(s)] [outproj(s-1)] [PV(s)] [chain(s)]; outproj(3) last
            with (
                tc.tile_pool(name="ps2", bufs=2, space="PSUM") as ps2,
                tc.tile_pool(name="pspv", bufs=1, space="PSUM") as pspv,
                tc.tile_pool(name="ps3", bufs=1, space="PSUM") as ps3,
            ):
                expm = {}

                def emit_scores(s):
                    kts = range(max(0, 4 * s - 2), min(NKT - 1, 4 * s + 3) + 1)
                    for kt in kts:
                        if (kt, 0) in expm:
                            continue
                        j0 = 128 * kt
                        w = min(WIN, S - j0)
                        for m in range(2):
                            pss = ps2.tile([128, 2, 512], F32, tag="sc")
                            for hf in range(2):
                                nc.tensor.matmul(
                                    pss[:, hf, 0:w],
                                    kdup[64 * hf:64 * hf + 64, bass.ts(kt, 128)],
                                    qT[m][64 * hf:64 * hf + 64, j0:j0 + w],
                                    start=True, stop=True)
                            et = ep.tile([128, 2, WIN], F16, tag="e")
                            nc.scalar.activation(et[:, :, 0:w], pss[:, :, 0:w],
                                                 AF.Exp, scale=float(SCALE))
                            mbc = bass.AP(tensor=mask_sb.tensor,
                                          offset=mask_sb.offset,
                                          ap=[list(mask_sb.ap[0]), [0, 2], [1, w]])
                            meng = nc.vector if m == 0 else nc.gpsimd
                            meng.tensor_tensor(out=et[:, :, 0:w], in0=et[:, :, 0:w],
                                               in1=mbc, op=OP.mult)
                            expm[(kt, m)] = et

                chain_in = {}

                def emit_pv(s):
                    pieces = _pv_pieces(s)
                    for hp in range(2):
                        pvp = pspv.tile([65, 2, 512], F32, tag="pv", name=f"pv{hp}_{s}")
                        for hh in range(2):
                            nc.tensor.matmul(pvp[:, hh, 0:1], zero65, zero65[:, 0:1],
                                             start=True, stop=False)
                        for i, (kt, c0, c1) in enumerate(pieces):
                            base = 128 * kt + c0 - 512 * s
                            last = i == len(pieces) - 1
                            for hh in range(2):
                                nc.tensor.matmul(pvp[:, hh, base:base + (c1 - c0)],
                                                 vaug[kt // 4][:, kt % 4, :],
                                                 expm[(kt, hp)][:, hh, c0:c1],
                                                 start=False, stop=last)
                        # evacuate PSUM immediately
                        pv_sb = npool.tile([64, 2, 512], F32, tag=f"pvs{hp}")
                        nc.vector.tensor_copy(pv_sb, pvp[0:64, :, :])
                        srow = npool.tile([1, 1024], F16, tag=f"srow{hp}")
                        nc.scalar.copy(srow, pvp[64:65, :, :])
                        chain_in[(s, hp)] = (pv_sb, srow)

                def emit_chain(s):
                    """Normalize chain for strip s; emitted one strip late so
                    every engine reaches these ops with inputs already ready.
                    Denominators broadcast via 1-contraction PE matmul, then
                    one reciprocal over all 64 partitions - no reshape DMAs."""
                    for hp in range(2):
                        pv_sb, srow = chain_in.pop((s, hp))
                        bcp = ps2.tile([128, 2, 512], F32, tag="sc",
                                       name=f"bc{s}_{hp}")
                        for hh in range(2):
                            nc.tensor.matmul(bcp[0:64, hh, :], ones1,
                                             srow[0:1, bass.ts(hh, 512)],
                                             start=True, stop=True)
                        rbc = npool.tile([64, 2, 512], F32, tag=f"rbc{hp}")
                        nc.vector.reciprocal_approx_fast(out=rbc,
                                                         in_=bcp[0:64, :, :])
                        nc.vector.tensor_tensor(
                            out=aT[hp][0:64, bass.ts(s, 512)],
                            in0=pv_sb[:, 0, :], in1=rbc[:, 0, :], op=OP.mult)
                        stag = npool.tile([64, 512], F16, tag=f"stag{hp}")
                        nc.vector.tensor_tensor(
                            out=stag, in0=pv_sb[:, 1, :],
                            in1=rbc[:, 1, :], op=OP.mult)
                        nc.sync.dma_start(out=aT[hp][64:128, bass.ts(s, 512)],
                                          in_=stag)

                def emit_outproj(s):
                    for st in range(4 * s, 4 * s + 4):
                        psos = ps3.tile([128, 2, 512], F32, tag="pso", name=f"pso{st}")
                        for ch in range(2):
                            for dd in range(2):
                                nc.tensor.matmul(psos[:, dd, :],
                                                 aT[ch][:, bass.ts(st, 128)],
                                                 wo_sb[:, ch, bass.ts(dd, 512)],
                                                 start=(ch == 0), stop=(ch == 1))
                        ost = opool.tile([128, 2, 512], F16, tag="o")
                        nc.vector.tensor_copy(ost[:, 0, :], psos[:, 0, :])
                        nc.scalar.copy(ost[:, 1, :], psos[:, 1, :])
                        nc.sync.dma_start(out=o_part[bass.ts(st, 128), :], in_=ost)

                for s in range(NST):
                    emit_scores(s)
                    if s > 0:
                        emit_chain(s - 1)
                    emit_pv(s)
                    if s > 0:
                        emit_outproj(s - 1)
                emit_chain(NST - 1)
                emit_outproj(NST - 1)

    nc.compile()
    return nc


_NC = {}


def _get_nc():
    if "nc" not in _NC:
        _NC["nc"] = _build()
    return _NC["nc"]


def _host_inputs(hidden_states, position_ids, Wq, Wk, Wv, Wo):
    hs = np.asarray(hidden_states, np.float32)
    Wq = np.asarray(Wq, np.float32)
    Wk = np.asarray(Wk, np.float32)
    Wv = np.asarray(Wv, np.float32)
    Wo = np.asarray(Wo, np.float32)

    hsT16 = [np.ascontiguousarray(hs[b].T).astype(np.float16).reshape(8, 128, S)
             for b in range(B)]

    inv_freq = (1.0 / (THETA ** (np.arange(0, HD, 2, dtype=np.float32) / HD))).astype(np.float32)
    cos2, sin2 = [], []
    for b in range(B):
        pos = np.asarray(position_ids[b]).astype(np.float32)
        freqs = pos[:, None] * inv_freq[None, :]          # [S, 32]
        cosf = np.cos(freqs).T                            # [32, S]
        sinf = np.sin(freqs).T
        cos64 = np.concatenate([cosf, cosf], axis=0)      # [64, S]
        sin64s = np.concatenate([-sinf, sinf], axis=0)    # sign-folded
        cos2.append(np.concatenate([cos64, cos64], axis=0).astype(np.float16))
        sin2.append(np.concatenate([sin64s, sin64s], axis=0).astype(np.float16))

    p = np.arange(128)[:, None]
    c = np.arange(WIN)[None, :]
    bandmask = ((p <= c) & (c < p + WINDOW)).astype(np.float16)

    perm = np.zeros((64, 64), np.float32)
    for i in range(32):
        perm[i, i + 32] = 1.0
        perm[i + 32, i] = 1.0
    perm2 = np.kron(np.eye(2, dtype=np.float32), perm)    # [128, 128]
    permT = np.ascontiguousarray(perm2.T).astype(np.float16)

    dup64 = np.tile(np.eye(64, dtype=np.float32), (1, 2)).astype(np.float16)  # [64, 128]

    in_maps = []
    for core in range(8):
        b, g = divmod(core, 4)
        wqT = np.ascontiguousarray(Wq[256 * g:256 * (g + 1)].T).astype(np.float16).reshape(8, 128, 256)
        wkT = np.ascontiguousarray(Wk[64 * g:64 * (g + 1)].T).astype(np.float16).reshape(8, 128, 64)
        wvT = np.ascontiguousarray(Wv[64 * g:64 * (g + 1)].T).astype(np.float16).reshape(8, 128, 64)
        wkvT = np.concatenate([wvT, wkT], axis=-1)        # [8, 128, 128]: V | K
        woT = np.ascontiguousarray(Wo[:, 256 * g:256 * (g + 1)].T).astype(np.float16).reshape(2, 128, D)
        in_maps.append({
            "hsT": hsT16[b], "wqT": wqT, "wkvT": wkvT, "woT": woT,
            "cos2": cos2[b], "sinS2": sin2[b],
            "bandmask": bandmask, "permT": permT, "dup64": dup64,
        })
    return in_maps


def run_spmd(hidden_states, attention_mask, position_ids, Wq, Wk, Wv, Wo, **spmd_kwargs):
    nc = _get_nc()
    in_maps = _host_inputs(hidden_states, position_ids, Wq, Wk, Wv, Wo)
    res = run_bass_kernel_spmd(nc, in_maps, list(range(8)), **spmd_kwargs)
    out = np.zeros((B, S, D), np.float32)
    for core in range(8):
        out[core // 4] += np.asarray(res.results[core]["o_part"], np.float32)
    return out, res


def kernel(hidden_states, attention_mask, position_ids, Wq, Wk, Wv, Wo):
    out, _ = run_spmd(hidden_states, attention_mask, position_ids, Wq, Wk, Wv, Wo)
    return out


# revision 24
# speedup vs baseline: 1.0985x; 1.0985x over previous
"""Mimi-style GQA attention (RoPE + 250-wide sliding causal window) on 8 TRN2 NeuronCores.

Sharding (per spec hint): core c handles batch b=c//4 and KV-head group g=c%4
(4 query heads + 1 KV head). Wq/Wk/Wv column-sharded, Wo row-sharded along the
head dim; each core emits a partial [S, D] output (f16); host sums the 4
partials per batch.

The PE executes its queue in order, so emission order is the schedule: phase 1
emits both projection sweeps' matmuls before any rope chain, and phase 2
software-pipelines strips ([scores(s)] [outproj(s-1)] [PV(s)] [chain(s)]) so
the normalize chain latency is covered by ready matmul work instead of
head-of-line blocking the PE (which re-throttles HAM).

Per-core pipeline (matmul inputs fp16, PSUM accumulation fp32):
  1. Fused [Wv|Wk] + Wq projections from host-pre-transposed hsT, dt-outer in
     two 2-strip sweeps over all 3 groups so the PE tracks the hsT DMA stream
     (split over both HWDGE rings). RoPE as raw*cos + (PERM @ raw)*sin_signed;
     K computed on partitions 64:128 and duplicated to both halves with a
     block-dup matmul.
  2. Banded attention, scoresT [k, q]: per (k-tile, head-pair) one
     [128, 2, 512] PSUM tile (bank-aligned), the two heads' matmuls run
     concurrently on PE row-groups (0,0)/(64,0); one Exp per pair; band mask
     multiply alternates DVE/GpSimd. PV accumulates per 512-wide q-strip into
     [65, 2, 512] PSUM (row 64 = softmax denominators via ones column); bank
     cleared by a 1-column start=True seed, evacuated to SBUF immediately.
  3. Normalize: reciprocal_approx_fast on a DMA-reshaped [16, 128] layout,
     DMA partition-broadcast (stride-0 source), DVE multiplies; row-sharded
     Wo projection emits the f16 partial output.
"""
import os
import sys

for _p in ("/opt/trn_rl_repo", "/root/.axon_site/_ro/trn_rl_repo"):
    if os.path.isdir(_p) and _p not in sys.path:
        sys.path.append(_p)

import numpy as np
import concourse.bass as bass
import concourse.mybir as mybir
import concourse.tile as tile
from concourse import bacc
from concourse.bass_utils import run_bass_kernel_spmd
from concourse.masks import make_identity

F32 = mybir.dt.float32
F16 = mybir.dt.float16
AF = mybir.ActivationFunctionType
OP = mybir.AluOpType

B, S, D = 2, 2048, 1024
H, HK, HD = 16, 4, 64
WINDOW = 250
SCALE = 1.0 / np.sqrt(HD)
THETA = 10000.0
NKT = S // 128          # 16 k-tiles
NST = S // 512          # 4 q-strips
WIN = 384               # padded per-k-tile q-window


def _pv_pieces(s):
    """PV pieces for q-strip s: list of (kt, c0, c1) window-column ranges."""
    out = []
    for kt in range(max(0, 4 * s - 2), min(NKT - 1, 4 * s + 3) + 1):
        j0 = 128 * kt
        w = min(WIN, S - j0)
        c_lo = max(0, 512 * s - j0)
        c_hi = min(w, 512 * (s + 1) - j0)
        if c_lo < c_hi:
            out.append((kt, c_lo, c_hi))
    return out


def _build():
    nc = bacc.Bacc(None, target_bir_lowering=False)

    hsT = nc.declare_dram_parameter("hsT", [8, 128, S], F16, isOutput=False)
    wq = nc.declare_dram_parameter("wqT", [8, 128, 256], F16, isOutput=False)
    wkv = nc.declare_dram_parameter("wkvT", [8, 128, 128], F16, isOutput=False)
    wo = nc.declare_dram_parameter("woT", [2, 128, D], F16, isOutput=False)
    cosd = nc.declare_dram_parameter("cos2", [128, S], F16, isOutput=False)
    sind = nc.declare_dram_parameter("sinS2", [128, S], F16, isOutput=False)
    maskd = nc.declare_dram_parameter("bandmask", [128, WIN], F16, isOutput=False)
    permd = nc.declare_dram_parameter("permT", [128, 128], F16, isOutput=False)
    dupd = nc.declare_dram_parameter("dup64", [64, 128], F16, isOutput=False)
    o_part = nc.declare_dram_parameter("o_part", [S, D], F16, isOutput=True)

    with tile.TileContext(nc) as tc:
        with (
            tc.tile_pool(name="persist", bufs=1) as pp,
            tc.tile_pool(name="work", bufs=4) as wk_pool,
            tc.tile_pool(name="expm", bufs=14) as ep,
            tc.tile_pool(name="norm", bufs=2) as npool,
            tc.tile_pool(name="ost", bufs=4) as opool,
        ):
            # ---- constants needing no DMA: warmup + exp-table preload ----
            warm = pp.tile([128, 512], F16, tag="warm")
            nc.vector.memset(warm, 0.0)
            nc.scalar.activation(warm[0:1, 0:8], warm[0:1, 0:8], AF.Exp, scale=1.0)
            zero65 = pp.tile([128, 65], F16, tag="zero65")
            nc.vector.memset(zero65, 0.0)
            ones1 = pp.tile([1, 64], F16, tag="ones1")
            nc.vector.memset(ones1, 1.0)
            ident = pp.tile([64, 64], F16, tag="ident")
            make_identity(nc, ident)

            # ---- DMA loads: small weights first, hsT split over both rings ----
            wq_sb = pp.tile([128, 8, 256], F16, tag="wq")
            wkv_sb = pp.tile([128, 8, 128], F16, tag="wkv")
            perm_sb = pp.tile([128, 128], F16, tag="perm")
            dup_sb = pp.tile([128, 128], F16, tag="dup")
            mask_sb = pp.tile([128, WIN], F16, tag="mask")
            ht = [pp.tile([128, S], F16, tag=f"ht{dt}", name=f"ht{dt}")
                  for dt in range(8)]
            # sync ring: wkv, ht0 first so group-0 dt-0 matmuls start earliest
            nc.sync.dma_start(out=wkv_sb, in_=wkv.rearrange("a p c -> p a c"))
            nc.sync.dma_start(out=ht[0][:, 0:1024], in_=hsT[0][:, 0:1024])
            nc.sync.dma_start(out=ht[0][:, 1024:2048], in_=hsT[0][:, 1024:2048])
            nc.sync.dma_start(out=wq_sb, in_=wq.rearrange("a p c -> p a c"))
            # scalar ring: small constants then odd ht tiles
            nc.scalar.dma_start(out=perm_sb, in_=permd[:, :])
            nc.scalar.dma_start(out=dup_sb[64:128, :], in_=dupd[:, :])
            nc.scalar.dma_start(out=mask_sb, in_=maskd[:, :])
            for dt in range(1, 8):
                eng = nc.sync if dt % 2 == 0 else nc.scalar
                eng.dma_start(out=ht[dt][:, 0:1024], in_=hsT[dt][:, 0:1024])
                eng.dma_start(out=ht[dt][:, 1024:2048], in_=hsT[dt][:, 1024:2048])
            wo_sb = pp.tile([128, 2, D], F16, tag="wo")
            nc.gpsimd.dma_start(out=wo_sb, in_=wo.rearrange("a p c -> p a c"))
            cos_sb = pp.tile([128, S], F16, tag="cos")
            sin_sb = pp.tile([128, S], F16, tag="sin")
            nc.gpsimd.dma_start(out=cos_sb, in_=cosd[:, :])
            nc.gpsimd.dma_start(out=sin_sb, in_=sind[:, :])

            qT = [pp.tile([128, S], F16, tag=f"qT{m}", name=f"qT{m}") for m in range(2)]
            kdup = pp.tile([128, S], F16, tag="kdup")
            vaug = [pp.tile([128, 4, 65], F16, tag=f"vaug{sp}", name=f"vaug{sp}")
                    for sp in range(NST)]
            aT = [pp.tile([128, S], F16, tag=f"aT{m}", name=f"aT{m}") for m in range(2)]

            # ---- phase 1: projections + rope ----
            # Emission: [sweepA MMs][A raw copies][sweepB MMs][A ropes][B copies][B ropes]
            with (
                tc.tile_pool(name="ps1", bufs=6, space="PSUM") as ps1,
                tc.tile_pool(name="ps1b", bufs=1, space="PSUM") as ps1b,
            ):
                wps = ps1b.tile([128, 512], F32, tag="scr", name="warmps")
                for _w in range(16):
                    nc.tensor.matmul(wps, warm[:, 0:128], warm,
                                     start=True, stop=True)

                def rope_q(m, strip, raw):
                    """qT[m][:, strip] = raw*cos + (PERM @ raw)*sin_signed."""
                    sl = bass.ts(strip, 512)
                    rot = ps1b.tile([128, 512], F32, tag="scr", name=f"rotq{m}_{strip}")
                    nc.tensor.matmul(rot, perm_sb, raw, start=True, stop=True)
                    t1 = wk_pool.tile([128, 512], F16, tag="t1q")
                    nc.vector.tensor_tensor(out=t1, in0=rot, in1=sin_sb[:, sl],
                                            op=OP.mult)
                    t2 = wk_pool.tile([128, 512], F16, tag="t2q")
                    nc.gpsimd.tensor_tensor(out=t2, in0=raw, in1=cos_sb[:, sl],
                                            op=OP.mult)
                    nc.vector.tensor_tensor(out=qT[m][:, sl], in0=t1, in1=t2,
                                            op=OP.add)

                def consume_kv(strip, raw):
                    """V rows 0:64 -> vaug transposes; K rows 64:128 -> rope -> kdup."""
                    sl = bass.ts(strip, 512)
                    # V: 4 transposes into one psum tile, one batched copy out
                    pvt = ps1b.tile([128, 256], F16, tag="vtr", name=f"pvt{strip}")
                    for i in range(4):
                        nc.tensor.transpose(pvt[:, bass.ts(i, 64)],
                                            raw[0:64, bass.ts(i, 128)], ident)
                    vt = vaug[strip]
                    nc.vector.tensor_copy(
                        vt[:, :, 0:64],
                        bass.AP(tensor=pvt.tensor, offset=pvt.offset,
                                ap=[list(pvt.ap[0]), [64, 4], [1, 64]]))
                    nc.vector.memset(vt[:, :, 64:65], 1.0)
                    # K rope on partitions 64:128
                    rot = ps1b.tile([128, 512], F32, tag="scr", name=f"rotk{strip}")
                    nc.tensor.matmul(rot[64:128, :], perm_sb[64:128, 64:128],
                                     raw[64:128, :], start=True, stop=True,
                                     tile_position=(64, 64))
                    t1 = wk_pool.tile([128, 512], F16, tag="t1k")
                    nc.vector.tensor_tensor(out=t1[64:128, :], in0=rot[64:128, :],
                                            in1=sin_sb[64:128, sl], op=OP.mult)
                    t2 = wk_pool.tile([128, 512], F16, tag="t2k")
                    nc.gpsimd.tensor_tensor(out=t2[64:128, :], in0=raw[64:128, :],
                                            in1=cos_sb[64:128, sl], op=OP.mult)
                    kroped = wk_pool.tile([128, 512], F16, tag="krop")
                    nc.vector.tensor_tensor(out=kroped[64:128, :], in0=t1[64:128, :],
                                            in1=t2[64:128, :], op=OP.add)
                    dps = ps1b.tile([128, 512], F32, tag="scr", name=f"dup{strip}")
                    nc.tensor.matmul(dps, dup_sb[64:128, :], kroped[64:128, :],
                                     start=True, stop=True, tile_position=(64, 0))
                    nc.scalar.copy(kdup[:, sl], dps)

                def lhs_of(g, dt):
                    if g == 0:
                        return wkv_sb[:, dt]
                    return wq_sb[:, dt, bass.ts(g - 1, 128)]

                raws = {}

                def emit_mms(sweep):
                    strips = (2 * sweep, 2 * sweep + 1)
                    pst = {(g, sp): ps1.tile([128, 512], F32, tag="proj",
                                             name=f"pj{g}_{sp}")
                           for g in range(3) for sp in strips}
                    for dt in range(8):
                        if sweep == 0 and dt >= 2:
                            # keep-warm filler while the hsT DMA stream catches up
                            fil = ps1b.tile([128, 512], F32, tag="scr",
                                            name=f"fil{dt}")
                            for _w in range(3):
                                nc.tensor.matmul(fil, warm[:, 0:128], warm,
                                                 start=True, stop=True)
                        for g in range(3):
                            for sp in strips:
                                nc.tensor.matmul(pst[(g, sp)], lhs_of(g, dt),
                                                 ht[dt][:, bass.ts(sp, 512)],
                                                 start=(dt == 0), stop=(dt == 7))
                    return pst

                def emit_copies(sweep, pst):
                    strips = (2 * sweep, 2 * sweep + 1)
                    for g in range(3):
                        for sp in strips:
                            raw = wk_pool.tile([128, 512], F16, tag="raw")
                            nc.scalar.copy(raw, pst[(g, sp)])
                            raws[(g, sp)] = raw

                def emit_ropes(sweep):
                    strips = (2 * sweep, 2 * sweep + 1)
                    for g in range(3):
                        for sp in strips:
                            if g == 0:
                                consume_kv(sp, raws[(g, sp)])
                            else:
                                rope_q(g - 1, sp, raws[(g, sp)])

                pstA = emit_mms(0)
                emit_copies(0, pstA)
                pstB = emit_mms(1)
                emit_ropes(0)
                emit_copies(1, pstB)
                emit_ropes(1)

            # ---- phases 2+3: banded attention + output projection ----
            # per strip: [scores(s)] [outproj(s-1)] [PV(s)] [chain(s)]; outproj(3) last
            with (
                tc.tile_pool(name="ps2", bufs=2, space="PSUM") as ps2,
                tc.tile_pool(name="pspv", bufs=1, space="PSUM") as pspv,
                tc.tile_pool(name="ps3", bufs=1, space="PSUM") as ps3,
            ):
                expm = {}

                def emit_scores(s):
                    kts = range(max(0, 4 * s - 2), min(NKT - 1, 4 * s + 3) + 1)
                    for kt in kts:
                        if (kt, 0) in expm:
                            continue
                        j0 = 128 * kt
                        w = min(WIN, S - j0)
                        for m in range(2):
                            pss = ps2.tile([128, 2, 512], F32, tag="sc")
                            for hf in range(2):
                                nc.tensor.matmul(
                                    pss[:, hf, 0:w],
                                    kdup[64 * hf:64 * hf + 64, bass.ts(kt, 128)],
                                    qT[m][64 * hf:64 * hf + 64, j0:j0 + w],
                                    start=True, stop=True)
                            et = ep.tile([128, 2, WIN], F16, tag="e")
                            nc.scalar.activation(et[:, :, 0:w], pss[:, :, 0:w],
                                                 AF.Exp, scale=float(SCALE))
                            mbc = bass.AP(tensor=mask_sb.tensor,
                                          offset=mask_sb.offset,
                                          ap=[list(mask_sb.ap[0]), [0, 2], [1, w]])
                            meng = nc.vector if m == 0 else nc.gpsimd
                            meng.tensor_tensor(out=et[:, :, 0:w], in0=et[:, :, 0:w],
                                               in1=mbc, op=OP.mult)
                            expm[(kt, m)] = et

                chain_in = {}

                def emit_pv(s):
                    pieces = _pv_pieces(s)
                    for hp in range(2):
                        pvp = pspv.tile([65, 2, 512], F32, tag="pv", name=f"pv{hp}_{s}")
                        for hh in range(2):
                            nc.tensor.matmul(pvp[:, hh, 0:1], zero65, zero65[:, 0:1],
                                             start=True, stop=False)
                        for i, (kt, c0, c1) in enumerate(pieces):
                            base = 128 * kt + c0 - 512 * s
                            last = i == len(pieces) - 1
                            for hh in range(2):
                                nc.tensor.matmul(pvp[:, hh, base:base + (c1 - c0)],
                                                 vaug[kt // 4][:, kt % 4, :],
                                                 expm[(kt, hp)][:, hh, c0:c1],
                                                 start=False, stop=last)
                        # evacuate PSUM immediately
                        pv_sb = npool.tile([64, 2, 512], F32, tag=f"pvs{hp}")
                        nc.vector.tensor_copy(pv_sb, pvp[0:64, :, :])
                        srow = npool.tile([1, 1024], F16, tag=f"srow{hp}")
                        nc.scalar.copy(srow, pvp[64:65, :, :])
                        chain_in[(s, hp)] = (pv_sb, srow)

                def emit_chain(s):
                    """Normalize chain for strip s; emitted one strip late so
                    every engine reaches these ops with inputs already ready.
                    Denominators broadcast via 1-contraction PE matmul, then
                    one reciprocal over all 64 partitions - no reshape DMAs."""
                    for hp in range(2):
                        pv_sb, srow = chain_in.pop((s, hp))
                        bcp = ps2.tile([128, 2, 512], F32, tag="sc",
                                       name=f"bc{s}_{hp}")
                        for hh in range(2):
                            nc.tensor.matmul(bcp[0:64, hh, :], ones1,
                                             srow[0:1, bass.ts(hh, 512)],
                                             start=True, stop=True)
                        rbc = npool.tile([64, 2, 512], F32, tag=f"rbc{hp}")
                        nc.vector.reciprocal_approx_fast(out=rbc,
                                                         in_=bcp[0:64, :, :])
                        nc.vector.tensor_tensor(
                            out=aT[hp][0:64, bass.ts(s, 512)],
                            in0=pv_sb[:, 0, :], in1=rbc[:, 0, :], op=OP.mult)
                        stag = npool.tile([64, 512], F16, tag=f"stag{hp}")
                        nc.vector.tensor_tensor(
                            out=stag, in0=pv_sb[:, 1, :],
                            in1=rbc[:, 1, :], op=OP.mult)
                        nc.sync.dma_start(out=aT[hp][64:128, bass.ts(s, 512)],
                                          in_=stag)

                def emit_outproj(s):
                    for st in range(4 * s, 4 * s + 4):
                        psos = ps3.tile([128, 2, 512], F32, tag="pso", name=f"pso{st}")
                        for ch in range(2):
                            for dd in range(2):
                                nc.tensor.matmul(psos[:, dd, :],
                                                 aT[ch][:, bass.ts(st, 128)],
                                                 wo_sb[:, ch, bass.ts(dd, 512)],
                                                 start=(ch == 0), stop=(ch == 1))
                        ost = opool.tile([128, 2, 512], F16, tag="o")
                        nc.vector.tensor_copy(ost[:, 0, :], psos[:, 0, :])
                        nc.scalar.copy(ost[:, 1, :], psos[:, 1, :])
                        nc.sync.dma_start(out=o_part[bass.ts(st, 128), :], in_=ost)

                for s in range(NST):
                    emit_scores(s)
                    emit_pv(s)
                    if s > 0:
                        emit_chain(s - 1)
                    if s > 1:
                        emit_outproj(s - 2)
                emit_chain(NST - 1)
                emit_outproj(NST - 2)
                emit_outproj(NST - 1)

    nc.compile()
    return nc


_NC = {}


def _get_nc():
    if "nc" not in _NC:
        _NC["nc"] = _build()
    return _NC["nc"]


def _host_inputs(hidden_states, position_ids, Wq, Wk, Wv, Wo):
    hs = np.asarray(hidden_states, np.float32)
    Wq = np.asarray(Wq, np.float32)
    Wk = np.asarray(Wk, np.float32)
    Wv = np.asarray(Wv, np.float32)
    Wo = np.asarray(Wo, np.float32)

    hsT16 = [np.ascontiguousarray(hs[b].T).astype(np.float16).reshape(8, 128, S)
             for b in range(B)]

    inv_freq = (1.0 / (THETA ** (np.arange(0, HD, 2, dtype=np.float32) / HD))).astype(np.float32)
    cos2, sin2 = [], []
    for b in range(B):
        pos = np.asarray(position_ids[b]).astype(np.float32)
        freqs = pos[:, None] * inv_freq[None, :]          # [S, 32]
        cosf = np.cos(freqs).T                            # [32, S]
        sinf = np.sin(freqs).T
        cos64 = np.concatenate([cosf, cosf], axis=0)      # [64, S]
        sin64s = np.concatenate([-sinf, sinf], axis=0)    # sign-folded
        cos2.append(np.concatenate([cos64, cos64], axis=0).astype(np.float16))
        sin2.append(np.concatenate([sin64s, sin64s], axis=0).astype(np.float16))

    p = np.arange(128)[:, None]
    c = np.arange(WIN)[None, :]
    bandmask = ((p <= c) & (c < p + WINDOW)).astype(np.float16)

    perm = np.zeros((64, 64), np.float32)
    for i in range(32):
        perm[i, i + 32] = 1.0
        perm[i + 32, i] = 1.0
    perm2 = np.kron(np.eye(2, dtype=np.float32), perm)    # [128, 128]
    permT = np.ascontiguousarray(perm2.T).astype(np.float16)

    dup64 = np.tile(np.eye(64, dtype=np.float32), (1, 2)).astype(np.float16)  # [64, 128]

    in_maps = []
    for core in range(8):
        b, g = divmod(core, 4)
        wqT = np.ascontiguousarray(Wq[256 * g:256 * (g + 1)].T).astype(np.float16).reshape(8, 128, 256)
        wkT = np.ascontiguousarray(Wk[64 * g:64 * (g + 1)].T).astype(np.float16).reshape(8, 128, 64)
        wvT = np.ascontiguousarray(Wv[64 * g:64 * (g + 1)].T).astype(np.float16).reshape(8, 128, 64)
        wkvT = np.concatenate([wvT, wkT], axis=-1)        # [8, 128, 128]: V | K
        woT = np.ascontiguousarray(Wo[:, 256 * g:256 * (g + 1)].T).astype(np.float16).reshape(2, 128, D)
        in_maps.append({
            "hsT": hsT16[b], "wqT": wqT, "wkvT": wkvT, "woT": woT,
            "cos2": cos2[b], "sinS2": sin2[b],
            "bandmask": bandmask, "permT": permT, "dup64": dup64,
        })
    return in_maps


def run_spmd(hidden_states, attention_mask, position_ids, Wq, Wk, Wv, Wo, **spmd_kwargs):
    nc = _get_nc()
    in_maps = _host_inputs(hidden_states, position_ids, Wq, Wk, Wv, Wo)
    res = run_bass_kernel_spmd(nc, in_maps, list(range(8)), **spmd_kwargs)
    out = np.zeros((B, S, D), np.float32)
    for core in range(8):
        out[core // 4] += np.asarray(res.results[core]["o_part"], np.float32)
    return out, res


def kernel(hidden_states, attention_mask, position_ids, Wq, Wk, Wv, Wo):
    out, _ = run_spmd(hidden_states, attention_mask, position_ids, Wq, Wk, Wv, Wo)
    return out
